# revision 43
# baseline (speedup 1.0000x reference)
"""Trainium2 Bass kernel for nn_EntropyLM (wavelet-coeff mixer + chunked MHA + output proj).

Data-parallel: 16 (batch x chunk) blocks, 2 per NeuronCore.  Heavy matmuls
(q/k/v projections, attention scores, PV, wo) run fp8e4m3 with DoubleRow perf
mode (256-deep contraction / instr at 0.5 cycles/row); trunk-critical matmuls
(coeff, mixer w1/w2, output projection) stay bf16.  All weights are SBUF-
resident.  Scaling plan (validated numerically in precheck.py):
  wq8/wk8/wv8/wo8 = fp8(16*w);  qT8/kT8 = fp8(psum) = 16*q / 16*k
  scores_psum = 256*s -> exp scale = HD^-0.5/256;  et8 = fp8(exp)
  vN8 = fp8(psum/4) = 4*v;  ones8 = 0.5 -> denominator D = 0.5*sum(et)
  ocat = pv_psum * recip(D) = 8*o (bf16);  otc8 = fp8(8*o)
  wo_psum = 128*(o@wo) -> res = wo_psum*(1/128) + mixed
Output LN: one batched Sqrt per chunk feeds a per-token 1/sigma applied at
the final projection's PSUM->SBUF copy (keeps the Act engine mostly in the
exp table; Gelu runs as one batched block per chunk).
"""

import numpy as np
import ml_dtypes

B, S, H, G, W = 4, 4096, 1024, 256, 8
CHUNK = 1024
NUM_HEADS = 4
HD = H // NUM_HEADS          # 256 per-head dim
HM = H // 2                  # 512 mixer hidden
N_CHUNKS = B * (S // CHUNK)  # 16 independent chunks
N_CORES = 8
CPC = N_CHUNKS // N_CORES    # 2 chunks per core
NT = CHUNK // 128            # 8 token tiles per chunk
KH = H // 128                # 8 feature tiles (H)
KM = HM // 128               # 4 feature tiles (HM)
EPS = 1e-5
BF16 = ml_dtypes.bfloat16
F8 = ml_dtypes.float8_e4m3   # TRN fp8e4: max normal 240 (matches ml_dtypes e4m3)

_COMPILED = None


def _build(debug=False):
    import concourse.bass as bass  # noqa: F401
    import concourse.tile as tile
    from concourse import bacc, mybir

    bf = mybir.dt.bfloat16
    f8 = mybir.dt.float8e4
    f32 = mybir.dt.float32
    Alu = mybir.AluOpType
    Act = mybir.ActivationFunctionType
    DR = mybir.MatmulPerfMode.DoubleRow

    nc = bacc.Bacc("TRN2", target_bir_lowering=False, debug=False,
                   enable_asserts=False, num_devices=N_CORES)

    # ---- DRAM tensors (per-core views; same NEFF on all 8 cores) ----
    xt = nc.dram_tensor("xt", [CPC, KH, 128, CHUNK], bf, kind="ExternalInput")
    kernT = nc.dram_tensor("kernt", [H, W], bf, kind="ExternalInput")
    w1a = nc.dram_tensor("w1a", [W + 1, HM], bf, kind="ExternalInput")
    gln = nc.dram_tensor("gln", [128, KM], f32, kind="ExternalInput")
    bln = nc.dram_tensor("bln", [128, KM], f32, kind="ExternalInput")
    w2 = nc.dram_tensor("w2", [HM, H], bf, kind="ExternalInput")
    b2c = nc.dram_tensor("b2c", [128, KH], f32, kind="ExternalInput")
    wq8 = nc.dram_tensor("wq8", [H, H], f8, kind="ExternalInput")
    wk8 = nc.dram_tensor("wk8", [H, H], f8, kind="ExternalInput")
    wv8 = nc.dram_tensor("wv8", [H, H], f8, kind="ExternalInput")
    wo8 = nc.dram_tensor("wo8", [H, H], f8, kind="ExternalInput")
    gw = nc.dram_tensor("gw", [H, G], bf, kind="ExternalInput")
    bw = nc.dram_tensor("bw", [128, G], f32, kind="ExternalInput")
    y = nc.dram_tensor("y", [CPC, CHUNK, G], f32, kind="ExternalOutput")
    dbg = {}
    if debug:
        for nm, shp, dt in [
            ("dcoef", [W + 1, CHUNK], bf),
            ("dhidT", [128, KM, CHUNK], bf),
            ("dmix8", [128, KH, CHUNK], f8),
            ("dmixN", [128, NT, H], bf),
            ("dq8", [128, KH, CHUNK], f8),
            ("dk8", [128, KH, CHUNK], f8),
            ("dv8", [128, NT, H], f8),
            ("det8", [128, NT, CHUNK], f8),
            ("docat", [128, NT, H], bf),
            ("dres", [128, NT, H], bf),
        ]:
            dbg[nm] = nc.dram_tensor(nm, shp, dt, kind="ExternalOutput")

    with tile.TileContext(nc) as tc:
        with (
            tc.tile_pool(name="wp", bufs=1) as wp,
            tc.tile_pool(name="ws", bufs=1) as ws,
            tc.tile_pool(name="sm", bufs=3) as sm,
            tc.tile_pool(name="ps", bufs=4, space="PSUM") as ps,
            tc.tile_pool(name="ps2", bufs=2, space="PSUM") as ps2,
        ):
            # ---------- persistent weights ----------
            kt_sb = wp.tile([128, KH, W], bf, tag="ktw")
            nc.sync.dma_start(kt_sb[:], kernT.ap().rearrange("(i p) w -> p i w", p=128))
            ones8_sb = wp.tile([128, 2, 1], f8, tag="ones8")
            nc.vector.memset(ones8_sb[:], 0.5)
            eps_sb = wp.tile([128, 1], f32, tag="eps")
            nc.vector.memset(eps_sb[:], EPS)
            w1a_sb = wp.tile([W + 1, HM], bf, tag="w1a")
            gln_sb = wp.tile([128, KM], f32, tag="gln")
            bln_sb = wp.tile([128, KM], f32, tag="bln")
            b2_sb = wp.tile([128, KH], f32, tag="b2")
            w2_sb = wp.tile([128, KM, H], bf, tag="w2w")
            wq_sb = wp.tile([128, KH, H], f8, tag="wqw")
            wk_sb = wp.tile([128, KH, H], f8, tag="wkw")
            wv_sb = wp.tile([128, KH, H], f8, tag="wvw")
            wo_sb = wp.tile([128, KH, H], f8, tag="wow")
            gw_sb = wp.tile([128, KH, G], bf, tag="gw")
            bw_sb = wp.tile([128, G], f32, tag="bw")

            def load_w8(dst, src):
                nc.sync.dma_start(dst[:], src.ap().rearrange("(i p) m -> p i m", p=128))

            def stage1(c):
                """wavelet coeffs for chunk c -> coef (bf16 [W+1, CHUNK])"""
                coef = ws.tile([W + 1, CHUNK], bf, tag="coef", bufs=2)
                nc.gpsimd.memset(coef[:, :], 1.0)  # row W = folded-bias ones row
                cps = ps2.tile([128, CHUNK], f32, tag="st")
                for ki in range(KH):
                    xki = ws.tile([128, CHUNK], bf, tag="xki", bufs=3)
                    nc.sync.dma_start(xki[:], xt.ap()[c, ki])
                    for n in range(2):
                        # each 512-token half accumulates in its own PSUM bank
                        nc.tensor.matmul(cps[:W, n * 512:(n + 1) * 512],
                                         kt_sb[:, ki, :],
                                         xki[:, n * 512:(n + 1) * 512],
                                         start=(ki == 0),
                                         stop=(ki == KH - 1))
                nc.scalar.copy(coef[:W, :], cps[:W, :])
                return coef

            def stage2(c, coef, hidT):
                """mixer hidden + LN + gelu -> hidT (feature-major bf16)"""
                for t in range(NT):
                    hps = ps.tile([128, 512], f32, tag="mm")
                    nc.tensor.matmul(hps[:], coef[:, t * 128:(t + 1) * 128],
                                     w1a_sb[:], start=True, stop=True)
                    st6 = sm.tile([128, 6], f32, tag="st6")
                    nc.vector.bn_stats(st6[:], hps[:])
                    mv = sm.tile([128, 2], f32, tag="mv")
                    nc.vector.bn_aggr(mv[:], st6[:])
                    sq = sm.tile([128, 1], f32, tag="sq")
                    nc.scalar.activation(sq[:], mv[:, 1:2], Act.Sqrt, bias=eps_sb[:])
                    iv = sm.tile([128, 1], f32, tag="iv")
                    nc.vector.reciprocal(iv[:], sq[:])
                    tmp = sm.tile([128, HM], bf, tag="mtmp")
                    nc.vector.tensor_scalar(tmp[:], hps[:], mv[:, 0:1], iv[:],
                                            op0=Alu.subtract, op1=Alu.mult)
                    nc.sync.dma_start_transpose(hidT[:, :, t * 128:(t + 1) * 128],
                                                tmp[:])
                for ki in range(KM):
                    nc.scalar.activation(hidT[:, ki, :], hidT[:, ki, :], Act.Gelu,
                                         scale=gln_sb[:, ki:ki + 1],
                                         bias=bln_sb[:, ki:ki + 1])

            def stage3(c, hidT, mix8, mixN):
                """w2 matmul -> mixT8 (fp8 qkv operand) + mixN (bf16 residual)"""
                for m in range(KH):
                    mrot = ws.tile([128, CHUNK], bf, tag="mrot", bufs=3)
                    for n in range(2):
                        mps = ps.tile([128, 512], f32, tag="mm")
                        for ki in range(KM):
                            nc.tensor.matmul(mps[:], w2_sb[:, ki, m * 128:(m + 1) * 128],
                                             hidT[:, ki, n * 512:(n + 1) * 512],
                                             start=(ki == 0), stop=(ki == KM - 1))
                        nc.vector.tensor_scalar(mrot[:, n * 512:(n + 1) * 512], mps[:],
                                                b2_sb[:, m:m + 1], None, op0=Alu.add)
                        if n == 0:
                            nc.scalar.activation(mix8[:, m, :512], mps[:],
                                                 Act.Identity, bias=b2_sb[:, m:m + 1])
                        else:
                            nc.vector.tensor_scalar(mix8[:, m, 512:], mps[:],
                                                    b2_sb[:, m:m + 1], None,
                                                    op0=Alu.add)
                    nc.sync.dma_start_transpose(mixN[:, :, m * 128:(m + 1) * 128],
                                                mrot[:])

            def stage4(c, mix8, q8, k8, v8):
                """fp8 DoubleRow q/k (feature-major out) + v (token-major out).
                q/k interleaved per m so early heads' tiles land first."""
                for m in range(KH):
                    for (dst, wsb, eng) in ((q8, wq_sb, "act"), (k8, wk_sb, "dve")):
                        for n in range(2):
                            qps = ps.tile([128, 512], f32, tag="mm")
                            for j in range(KH // 2):
                                nc.tensor.matmul(
                                    qps[:], wsb[:, 2 * j:2 * j + 2, m * 128:(m + 1) * 128],
                                    mix8[:, 2 * j:2 * j + 2, n * 512:(n + 1) * 512],
                                    start=(j == 0), stop=(j == KH // 2 - 1),
                                    perf_mode=DR)
                            if eng == "act":
                                nc.scalar.copy(dst[:, m, n * 512:(n + 1) * 512], qps[:])
                            else:
                                nc.vector.tensor_copy(dst[:, m, n * 512:(n + 1) * 512],
                                                      qps[:])
                for t in range(NT):
                    for n in range(2):
                        vps = ps.tile([128, 512], f32, tag="mm")
                        for j in range(KH // 2):
                            nc.tensor.matmul(
                                vps[:], mix8[:, 2 * j:2 * j + 2, t * 128:(t + 1) * 128],
                                wv_sb[:, 2 * j:2 * j + 2, n * 512:(n + 1) * 512],
                                start=(j == 0), stop=(j == KH // 2 - 1),
                                perf_mode=DR)
                        nc.vector.tensor_scalar(v8[:, t, n * 512:(n + 1) * 512], vps[:],
                                                0.25, None, op0=Alu.mult)

            def scores_head(c, h, q8, k8):
                """scores + exp for one head -> et8 fp8 [128, NT, CHUNK]"""
                et8 = ws.tile([128, NT, CHUNK], f8, tag="et8", bufs=2)
                for kt in range(NT):
                    stp = ps2.tile([128, CHUNK], f32, tag="st")
                    for qn in range(2):
                        nc.tensor.matmul(
                            stp[:, qn * 512:(qn + 1) * 512],
                            k8[:, 2 * h:2 * h + 2, kt * 128:(kt + 1) * 128],
                            q8[:, 2 * h:2 * h + 2, qn * 512:(qn + 1) * 512],
                            start=True, stop=True, perf_mode=DR)
                    nc.scalar.activation(et8[:, kt, :], stp[:], Act.Exp,
                                         scale=float(HD ** -0.5 / 256.0))
                return et8

            def pv_head(c, h, et8, v8, ocat, norm_eng="dve"):
                """PV + denominator + normalize -> ocat[:, qt, h*HD:(h+1)*HD]"""
                for qt in range(NT):
                    ovp = ps.tile([128, 512], f32, tag="mm")
                    for j in range(NT // 2):
                        nc.tensor.matmul(ovp[:, :HD],
                                         et8[:, 2 * j:2 * j + 2, qt * 128:(qt + 1) * 128],
                                         v8[:, 2 * j:2 * j + 2, h * HD:(h + 1) * HD],
                                         start=(j == 0), stop=(j == NT // 2 - 1),
                                         perf_mode=DR)
                        # denominator column in the same PSUM bank; start=False
                        # always (j==0 PV's bank-clear leaves has_written=0 so
                        # the first write overwrites, later ones accumulate).
                        nc.tensor.matmul(ovp[:, HD:HD + 1],
                                         et8[:, 2 * j:2 * j + 2, qt * 128:(qt + 1) * 128],
                                         ones8_sb[:],
                                         start=False, stop=(j == NT // 2 - 1),
                                         perf_mode=DR, skip_group_check=True)
                    rq = sm.tile([128, 1], f32, tag="rq")
                    nc.vector.reciprocal(rq[:], ovp[:, HD:HD + 1])
                    if norm_eng == "dve":
                        nc.vector.tensor_scalar(ocat[:, qt, h * HD:(h + 1) * HD],
                                                ovp[:, :HD], rq[:], None, op0=Alu.mult)
                    else:
                        nc.scalar.activation(ocat[:, qt, h * HD:(h + 1) * HD],
                                             ovp[:, :HD], Act.Identity, scale=rq[:])

            def tail_p1(c, qt, ocat, mixN, res, mvA, cast_eng="pool",
                        pair_ps=False):
                """transpose o, wo matmul, residual into res, LN stats."""
                otcb = ws.tile([128, KH, 128], bf, tag="sm2b", bufs=4)
                nc.sync.dma_start_transpose(otcb[:], ocat[:, qt, :])
                otc8 = ws.tile([128, KH, 128], f8, tag="otc8", bufs=4)
                if cast_eng == "pool":
                    nc.gpsimd.tensor_copy(otc8[:], otcb[:])
                else:
                    nc.scalar.copy(otc8[:], otcb[:])
                if pair_ps:
                    # both wo halves into one 2-bank ps2 tile -> single wide stt
                    wps = ps2.tile([128, CHUNK], f32, tag="st")
                    for n in range(2):
                        for j in range(KH // 2):
                            nc.tensor.matmul(
                                wps[:, n * 512:(n + 1) * 512],
                                otc8[:, 2 * j:2 * j + 2, :],
                                wo_sb[:, 2 * j:2 * j + 2, n * 512:(n + 1) * 512],
                                start=(j == 0), stop=(j == KH // 2 - 1),
                                perf_mode=DR)
                    nc.vector.scalar_tensor_tensor(
                        res[:, qt, :], wps[:], 1.0 / 128.0,
                        mixN[:, qt, :], op0=Alu.mult, op1=Alu.add)
                else:
                    for n in range(2):
                        ops_ = ps.tile([128, 512], f32, tag="mm")
                        for j in range(KH // 2):
                            nc.tensor.matmul(ops_[:],
                                             otc8[:, 2 * j:2 * j + 2, :],
                                             wo_sb[:, 2 * j:2 * j + 2, n * 512:(n + 1) * 512],
                                             start=(j == 0), stop=(j == KH // 2 - 1),
                                             perf_mode=DR)
                        nc.vector.scalar_tensor_tensor(
                            res[:, qt, n * 512:(n + 1) * 512], ops_[:], 1.0 / 128.0,
                            mixN[:, qt, n * 512:(n + 1) * 512],
                            op0=Alu.mult, op1=Alu.add)
                st6 = sm.tile([128, 2, 6], f32, tag="st6b")
                for half in range(2):
                    nc.vector.bn_stats(st6[:, half, :],
                                       res[:, qt, half * 512:(half + 1) * 512])
                nc.vector.bn_aggr(mvA[:, qt, :], st6[:])

            def tail_iv(c, mvA, ivA, negA):
                """batched rsqrt(var+eps) for the whole chunk (one Sqrt op)."""
                sqA = sm.tile([128, NT], f32, tag="sqA")
                nc.scalar.activation(sqA[:], mvA[:, :, 1], Act.Sqrt, bias=eps_sb[:])
                nc.vector.reciprocal(ivA[:], sqA[:])
                nc.vector.tensor_scalar(negA[:], ivA[:], -1.0, None, op0=Alu.mult)

            def tail_p2(c, qt, res, mvA, ivA, negA, ych, z_eng="pool"):
                """z = res - mean; y = (z @ gw) * iv + bw."""
                z = ws.tile([128, CHUNK], bf, tag="sm2b", bufs=4, name="z")
                if z_eng == "pool":
                    nc.gpsimd.tensor_scalar(z[:], res[:, qt, :], mvA[:, qt, 0:1], None,
                                            op0=Alu.subtract)
                else:
                    # z~ = mean - res; compensated by negated iv in the stt
                    nc.scalar.activation(z[:], res[:, qt, :], Act.Identity,
                                         bias=mvA[:, qt, 0:1], scale=-1.0)
                zT = ws.tile([128, KH, 128], bf, tag="sm2b", bufs=4, name="zT")
                nc.sync.dma_start_transpose(zT[:], z[:])
                yps = ps.tile([128, 512], f32, tag="mm")
                for fi in range(KH):
                    nc.tensor.matmul(yps[:, :G], zT[:, fi, :], gw_sb[:, fi, :],
                                     start=(fi == 0), stop=(fi == KH - 1))
                dv = negA if z_eng != "pool" else ivA
                nc.vector.scalar_tensor_tensor(ych[:, qt, :], yps[:, :G],
                                               dv[:, qt:qt + 1], bw_sb[:],
                                               op0=Alu.mult, op1=Alu.add)

            def store(c, ych, t0, t1):
                nc.sync.dma_start(
                    y.ap()[c, t0 * 128:t1 * 128, :].rearrange(
                        "(t p) g -> p t g", p=128),
                    ych[:, t0:t1, :])

            # =================== emission schedule ===================
            coef0 = stage1(0)
            nc.sync.dma_start(w1a_sb[:], w1a.ap())
            nc.sync.dma_start(gln_sb[:], gln.ap())
            nc.sync.dma_start(bln_sb[:], bln.ap())
            coef1 = stage1(1)
            nc.sync.dma_start(b2_sb[:], b2c.ap())
            nc.sync.dma_start(w2_sb[:], w2.ap().rearrange("(i p) m -> p i m", p=128))

            hidT0 = ws.tile([128, KM, CHUNK], bf, tag="hidT", bufs=1)
            stage2(0, coef0, hidT0)
            load_w8(wq_sb, wq8)
            load_w8(wk_sb, wk8)
            mix8_0 = ws.tile([128, KH, CHUNK], f8, tag="mix8", bufs=1)
            mixN0 = ws.tile([128, NT, H], bf, tag="mixN", bufs=2)
            stage3(0, hidT0, mix8_0, mixN0)
            load_w8(wv_sb, wv8)

            q8_0 = ws.tile([128, KH, CHUNK], f8, tag="q8", bufs=1)
            k8_0 = ws.tile([128, KH, CHUNK], f8, tag="k8", bufs=1)
            v8_0 = ws.tile([128, NT, H], f8, tag="v8y", bufs=2)
            stage4(0, mix8_0, q8_0, k8_0, v8_0)
            if debug:
                nc.sync.dma_start(dbg["dcoef"].ap(), coef0[:])
                nc.sync.dma_start(dbg["dhidT"].ap(), hidT0[:])
                nc.sync.dma_start(dbg["dmix8"].ap(), mix8_0[:])
                nc.sync.dma_start(dbg["dmixN"].ap(), mixN0[:])
                nc.sync.dma_start(dbg["dq8"].ap(), q8_0[:])
                nc.sync.dma_start(dbg["dk8"].ap(), k8_0[:])
                nc.sync.dma_start(dbg["dv8"].ap(), v8_0[:])

            # --- c0 attention interleaved with c1 front stages ---
            ocat0 = ws.tile([128, NT, H], bf, tag="ocat", bufs=1)
            et_a = scores_head(0, 0, q8_0, k8_0)
            load_w8(wo_sb, wo8)
            nc.sync.dma_start(gw_sb[:], gw.ap().rearrange("(i p) g -> p i g", p=128))
            nc.sync.dma_start(bw_sb[:], bw.ap())
            hidT1 = ws.tile([128, KM, CHUNK], bf, tag="hidT", bufs=1)
            stage2(1, coef1, hidT1)
            et_b = scores_head(0, 1, q8_0, k8_0)
            pv_head(0, 0, et_a, v8_0, ocat0)
            if debug:
                nc.sync.dma_start(dbg["det8"].ap(), et_b[:])
            mix8_1 = ws.tile([128, KH, CHUNK], f8, tag="mix8", bufs=1)
            mixN1 = ws.tile([128, NT, H], bf, tag="mixN", bufs=2)
            stage3(1, hidT1, mix8_1, mixN1)
            et_c = scores_head(0, 2, q8_0, k8_0)
            pv_head(0, 1, et_b, v8_0, ocat0)
            et_d = scores_head(0, 3, q8_0, k8_0)
            q8_1 = ws.tile([128, KH, CHUNK], f8, tag="q8", bufs=1)
            k8_1 = ws.tile([128, KH, CHUNK], f8, tag="k8", bufs=1)
            v8_1 = ws.tile([128, NT, H], f8, tag="v8y", bufs=2)
            stage4(1, mix8_1, q8_1, k8_1, v8_1)
            pv_head(0, 2, et_c, v8_0, ocat0)

            ych0 = ws.tile([128, NT, G], f32, tag="v8y", bufs=2, name="ych")
            et_e = scores_head(1, 0, q8_1, k8_1)
            pv_head(0, 3, et_d, v8_0, ocat0)
            if debug:
                nc.sync.dma_start(dbg["docat"].ap(), ocat0[:])
            et_f = scores_head(1, 1, q8_1, k8_1)

            # --- c0 tails (two passes) with c1 PV interleaved ---
            res0 = ws.tile([128, NT, H], bf, tag="res", bufs=1)
            mvA0 = sm.tile([128, NT, 2], f32, tag="mvA", bufs=2)
            ivA0 = sm.tile([128, NT], f32, tag="ivA", bufs=2)
            negA0 = sm.tile([128, NT], f32, tag="negA", bufs=2)
            ocat1 = ws.tile([128, NT, H], bf, tag="ocat", bufs=1, name="ocat")
            for qt in range(4):
                tail_p1(0, qt, ocat0, mixN0, res0, mvA0, cast_eng="pool",
                        pair_ps=True)
            pv_head(1, 0, et_e, v8_1, ocat1)
            for qt in range(4, NT):
                tail_p1(0, qt, ocat0, mixN0, res0, mvA0, cast_eng="pool",
                        pair_ps=True)
            if debug:
                nc.sync.dma_start(dbg["dres"].ap(), res0[:])
            tail_iv(0, mvA0, ivA0, negA0)
            et_g = scores_head(1, 2, q8_1, k8_1)
            for qt in range(4):
                tail_p2(0, qt, res0, mvA0, ivA0, negA0, ych0, z_eng="pool")
            store(0, ych0, 0, 4)
            pv_head(1, 1, et_f, v8_1, ocat1)
            for qt in range(4, NT):
                tail_p2(0, qt, res0, mvA0, ivA0, negA0, ych0, z_eng="pool")
            store(0, ych0, 4, NT)
            et_h = scores_head(1, 3, q8_1, k8_1)
            pv_head(1, 2, et_g, v8_1, ocat1, norm_eng="act")

            # --- c1 tails (Act engine is idle here; use it) ---
            res1 = ws.tile([128, NT, H], bf, tag="res", bufs=1)
            mvA1 = sm.tile([128, NT, 2], f32, tag="mvA", bufs=2)
            ivA1 = sm.tile([128, NT], f32, tag="ivA", bufs=2)
            negA1 = sm.tile([128, NT], f32, tag="negA", bufs=2)
            ych1 = ws.tile([128, NT, G], f32, tag="v8y", bufs=2, name="ych")
            pv_head(1, 3, et_h, v8_1, ocat1, norm_eng="act")
            for qt in range(NT):
                tail_p1(1, qt, ocat1, mixN1, res1, mvA1, cast_eng="pool",
                        pair_ps=True)
            tail_iv(1, mvA1, ivA1, negA1)
            for qt in range(NT):
                tail_p2(1, qt, res1, mvA1, ivA1, negA1, ych1, z_eng="act")
                if qt == 3:
                    store(1, ych1, 0, 4)
            store(1, ych1, 4, NT)

    nc.compile()
    return nc


def _get_compiled(debug=False):
    global _COMPILED
    if _COMPILED is None:
        _COMPILED = _build(debug=debug)
    return _COMPILED


def _f8c(x):
    return np.clip(x, -240.0, 240.0).astype(F8)


def _prep_inputs(inputs):
    f32 = np.float32

    def a(name):
        return np.asarray(inputs[name], dtype=f32)

    x = a("x")
    mw = a("mother_wavelets")
    scales = a("scales")
    norm = np.sqrt(np.sum(mw ** 2, axis=2, keepdims=True))
    kern = (mw / np.maximum(norm, 1e-12)) * (1.0 / (1.0 + np.exp(-scales)))
    kern = kern[0, :, :, 0]                      # (W, H)
    kernT = np.ascontiguousarray(kern.T).astype(BF16)

    w1a = np.concatenate([a("mix_w1"), a("mix_b1")[None, :]], axis=0).astype(BF16)
    gln = np.ascontiguousarray(a("mix_ln_g").reshape(KM, 128).T).astype(f32)
    bln = np.ascontiguousarray(a("mix_ln_b").reshape(KM, 128).T).astype(f32)
    w2 = a("mix_w2").astype(BF16)
    b2c = np.ascontiguousarray(a("mix_b2").reshape(KH, 128).T).astype(f32)
    gw = (a("out_ln_g")[:, None] * a("out_w")).astype(BF16)
    bw_vec = a("out_ln_b") @ a("out_w") + a("out_b")
    bw = np.tile(bw_vec[None, :], (128, 1)).astype(f32)

    shared = {
        "kernt": kernT, "w1a": w1a, "gln": gln, "bln": bln, "w2": w2,
        "b2c": b2c,
        "wq8": _f8c(16.0 * a("wq")), "wk8": _f8c(16.0 * a("wk")),
        "wv8": _f8c(16.0 * a("wv")), "wo8": _f8c(16.0 * a("wo")),
        "gw": gw, "bw": bw,
    }

    xc = x.reshape(N_CHUNKS, CHUNK, H)
    # xt[c, ki, p, t] = xc[c, t, ki*128+p]  (feature-major per 128-slice)
    xt_all = np.ascontiguousarray(
        xc.transpose(0, 2, 1).reshape(N_CHUNKS, KH, 128, CHUNK)).astype(BF16)
    in_maps = []
    for core in range(N_CORES):
        m = dict(shared)
        m["xt"] = np.ascontiguousarray(xt_all[core * CPC:(core + 1) * CPC])
        in_maps.append(m)
    return in_maps


def kernel(**inputs) -> np.ndarray:
    from concourse.bass_utils import run_bass_kernel_spmd

    nc = _get_compiled()
    in_maps = _prep_inputs(inputs)
    res = run_bass_kernel_spmd(nc, in_maps, core_ids=list(range(N_CORES)))
    out = np.concatenate([r["y"] for r in res.results], axis=0)  # (16, CHUNK, G)
    return out.reshape(B, S, G).astype(np.float32)


# revision 44
# speedup vs baseline: 1.0007x; 1.0007x over previous
"""Trainium2 Bass kernel for nn_EntropyLM (wavelet-coeff mixer + chunked MHA + output proj).

Data-parallel: 16 (batch x chunk) blocks, 2 per NeuronCore.  Heavy matmuls
(q/k/v projections, attention scores, PV, wo) run fp8e4m3 with DoubleRow perf
mode (256-deep contraction / instr at 0.5 cycles/row); trunk-critical matmuls
(coeff, mixer w1/w2, output projection) stay bf16.  All weights are SBUF-
resident.  Scaling plan (validated numerically in precheck.py):
  wq8/wk8/wv8/wo8 = fp8(16*w);  qT8/kT8 = fp8(psum) = 16*q / 16*k
  scores_psum = 256*s -> exp scale = HD^-0.5/256;  et8 = fp8(exp)
  vN8 = fp8(psum/4) = 4*v;  ones8 = 0.5 -> denominator D = 0.5*sum(et)
  ocat = pv_psum * recip(D) = 8*o (bf16);  otc8 = fp8(8*o)
  wo_psum = 128*(o@wo) -> res = wo_psum*(1/128) + mixed
Output LN: one batched Sqrt per chunk feeds a per-token 1/sigma applied at
the final projection's PSUM->SBUF copy (keeps the Act engine mostly in the
exp table; Gelu runs as one batched block per chunk).
"""

import numpy as np
import ml_dtypes

B, S, H, G, W = 4, 4096, 1024, 256, 8
CHUNK = 1024
NUM_HEADS = 4
HD = H // NUM_HEADS          # 256 per-head dim
HM = H // 2                  # 512 mixer hidden
N_CHUNKS = B * (S // CHUNK)  # 16 independent chunks
N_CORES = 8
CPC = N_CHUNKS // N_CORES    # 2 chunks per core
NT = CHUNK // 128            # 8 token tiles per chunk
KH = H // 128                # 8 feature tiles (H)
KM = HM // 128               # 4 feature tiles (HM)
EPS = 1e-5
BF16 = ml_dtypes.bfloat16
F8 = ml_dtypes.float8_e4m3   # TRN fp8e4: max normal 240 (matches ml_dtypes e4m3)

_COMPILED = None


def _build(debug=False):
    import concourse.bass as bass  # noqa: F401
    import concourse.tile as tile
    from concourse import bacc, mybir

    bf = mybir.dt.bfloat16
    f8 = mybir.dt.float8e4
    f32 = mybir.dt.float32
    Alu = mybir.AluOpType
    Act = mybir.ActivationFunctionType
    DR = mybir.MatmulPerfMode.DoubleRow

    nc = bacc.Bacc("TRN2", target_bir_lowering=False, debug=False,
                   enable_asserts=False, num_devices=N_CORES)

    # ---- DRAM tensors (per-core views; same NEFF on all 8 cores) ----
    xt = nc.dram_tensor("xt", [CPC, KH, 128, CHUNK], bf, kind="ExternalInput")
    kernT = nc.dram_tensor("kernt", [H, W], bf, kind="ExternalInput")
    w1a = nc.dram_tensor("w1a", [W + 1, HM], bf, kind="ExternalInput")
    gln = nc.dram_tensor("gln", [128, KM], f32, kind="ExternalInput")
    bln = nc.dram_tensor("bln", [128, KM], f32, kind="ExternalInput")
    w2 = nc.dram_tensor("w2", [HM, H], bf, kind="ExternalInput")
    b2c = nc.dram_tensor("b2c", [128, KH], f32, kind="ExternalInput")
    wq8 = nc.dram_tensor("wq8", [H, H], f8, kind="ExternalInput")
    wk8 = nc.dram_tensor("wk8", [H, H], f8, kind="ExternalInput")
    wv8 = nc.dram_tensor("wv8", [H, H], f8, kind="ExternalInput")
    wo8 = nc.dram_tensor("wo8", [H, H], f8, kind="ExternalInput")
    gw = nc.dram_tensor("gw", [H, G], bf, kind="ExternalInput")
    bw = nc.dram_tensor("bw", [128, G], f32, kind="ExternalInput")
    y = nc.dram_tensor("y", [CPC, CHUNK, G], f32, kind="ExternalOutput")
    dbg = {}
    if debug:
        for nm, shp, dt in [
            ("dcoef", [W + 1, CHUNK], bf),
            ("dhidT", [128, KM, CHUNK], bf),
            ("dmix8", [128, KH, CHUNK], f8),
            ("dmixN", [128, NT, H], bf),
            ("dq8", [128, KH, CHUNK], f8),
            ("dk8", [128, KH, CHUNK], f8),
            ("dv8", [128, NT, H], f8),
            ("det8", [128, NT, CHUNK], f8),
            ("docat", [128, NT, H], bf),
            ("dres", [128, NT, H], bf),
        ]:
            dbg[nm] = nc.dram_tensor(nm, shp, dt, kind="ExternalOutput")

    with tile.TileContext(nc) as tc:
        with (
            tc.tile_pool(name="wp", bufs=1) as wp,
            tc.tile_pool(name="ws", bufs=1) as ws,
            tc.tile_pool(name="sm", bufs=3) as sm,
            tc.tile_pool(name="ps", bufs=4, space="PSUM") as ps,
            tc.tile_pool(name="ps2", bufs=2, space="PSUM") as ps2,
        ):
            # ---------- persistent weights ----------
            kt_sb = wp.tile([128, KH, W], bf, tag="ktw")
            nc.sync.dma_start(kt_sb[:], kernT.ap().rearrange("(i p) w -> p i w", p=128))
            ones8_sb = wp.tile([128, 2, 1], f8, tag="ones8")
            nc.vector.memset(ones8_sb[:], 0.5)
            eps_sb = wp.tile([128, 1], f32, tag="eps")
            nc.vector.memset(eps_sb[:], EPS)
            w1a_sb = wp.tile([W + 1, HM], bf, tag="w1a")
            gln_sb = wp.tile([128, KM], f32, tag="gln")
            bln_sb = wp.tile([128, KM], f32, tag="bln")
            b2_sb = wp.tile([128, KH], f32, tag="b2")
            w2_sb = wp.tile([128, KM, H], bf, tag="w2w")
            wq_sb = wp.tile([128, KH, H], f8, tag="wqw")
            wk_sb = wp.tile([128, KH, H], f8, tag="wkw")
            wv_sb = wp.tile([128, KH, H], f8, tag="wvw")
            wo_sb = wp.tile([128, KH, H], f8, tag="wow")
            gw_sb = wp.tile([128, KH, G], bf, tag="gw")
            bw_sb = wp.tile([128, G], f32, tag="bw")

            def load_w8(dst, src):
                nc.sync.dma_start(dst[:], src.ap().rearrange("(i p) m -> p i m", p=128))

            def stage1(c):
                """wavelet coeffs for chunk c -> coef (bf16 [W+1, CHUNK])"""
                coef = ws.tile([W + 1, CHUNK], bf, tag="coef", bufs=2)
                nc.gpsimd.memset(coef[:, :], 1.0)  # row W = folded-bias ones row
                cps = ps2.tile([128, CHUNK], f32, tag="st")
                for ki in range(KH):
                    xki = ws.tile([128, CHUNK], bf, tag="xki", bufs=3)
                    nc.sync.dma_start(xki[:], xt.ap()[c, ki])
                    for n in range(2):
                        # each 512-token half accumulates in its own PSUM bank
                        nc.tensor.matmul(cps[:W, n * 512:(n + 1) * 512],
                                         kt_sb[:, ki, :],
                                         xki[:, n * 512:(n + 1) * 512],
                                         start=(ki == 0),
                                         stop=(ki == KH - 1))
                nc.scalar.copy(coef[:W, :], cps[:W, :])
                return coef

            def stage2(c, coef, hidT):
                """mixer hidden + LN + gelu -> hidT (feature-major bf16)"""
                for t in range(NT):
                    hps = ps.tile([128, 512], f32, tag="mm")
                    nc.tensor.matmul(hps[:], coef[:, t * 128:(t + 1) * 128],
                                     w1a_sb[:], start=True, stop=True)
                    st6 = sm.tile([128, 6], f32, tag="st6")
                    nc.vector.bn_stats(st6[:], hps[:])
                    mv = sm.tile([128, 2], f32, tag="mv")
                    nc.vector.bn_aggr(mv[:], st6[:])
                    sq = sm.tile([128, 1], f32, tag="sq")
                    nc.scalar.activation(sq[:], mv[:, 1:2], Act.Sqrt, bias=eps_sb[:])
                    iv = sm.tile([128, 1], f32, tag="iv")
                    nc.vector.reciprocal(iv[:], sq[:])
                    tmp = sm.tile([128, HM], bf, tag="mtmp")
                    nc.vector.tensor_scalar(tmp[:], hps[:], mv[:, 0:1], iv[:],
                                            op0=Alu.subtract, op1=Alu.mult)
                    nc.sync.dma_start_transpose(hidT[:, :, t * 128:(t + 1) * 128],
                                                tmp[:])
                for ki in range(KM):
                    nc.scalar.activation(hidT[:, ki, :], hidT[:, ki, :], Act.Gelu,
                                         scale=gln_sb[:, ki:ki + 1],
                                         bias=bln_sb[:, ki:ki + 1])

            def stage3(c, hidT, mix8, mixN):
                """w2 matmul -> mixT8 (fp8 qkv operand) + mixN (bf16 residual)"""
                for m in range(KH):
                    mrot = ws.tile([128, CHUNK], bf, tag="mrot", bufs=3)
                    for n in range(2):
                        mps = ps.tile([128, 512], f32, tag="mm")
                        for ki in range(KM):
                            nc.tensor.matmul(mps[:], w2_sb[:, ki, m * 128:(m + 1) * 128],
                                             hidT[:, ki, n * 512:(n + 1) * 512],
                                             start=(ki == 0), stop=(ki == KM - 1))
                        nc.vector.tensor_scalar(mrot[:, n * 512:(n + 1) * 512], mps[:],
                                                b2_sb[:, m:m + 1], None, op0=Alu.add)
                        if n == 0:
                            nc.scalar.activation(mix8[:, m, :512], mps[:],
                                                 Act.Identity, bias=b2_sb[:, m:m + 1])
                        else:
                            nc.vector.tensor_scalar(mix8[:, m, 512:], mps[:],
                                                    b2_sb[:, m:m + 1], None,
                                                    op0=Alu.add)
                    nc.sync.dma_start_transpose(mixN[:, :, m * 128:(m + 1) * 128],
                                                mrot[:])

            def stage4_qk(c, mix8, q8, k8, ms):
                """fp8 DoubleRow q/k (feature-major out), m-tiles ms."""
                for m in ms:
                    for (dst, wsb, eng) in ((q8, wq_sb, "act"), (k8, wk_sb, "dve")):
                        for n in range(2):
                            qps = ps.tile([128, 512], f32, tag="mm")
                            for j in range(KH // 2):
                                nc.tensor.matmul(
                                    qps[:], wsb[:, 2 * j:2 * j + 2, m * 128:(m + 1) * 128],
                                    mix8[:, 2 * j:2 * j + 2, n * 512:(n + 1) * 512],
                                    start=(j == 0), stop=(j == KH // 2 - 1),
                                    perf_mode=DR)
                            if eng == "act":
                                nc.scalar.copy(dst[:, m, n * 512:(n + 1) * 512], qps[:])
                            else:
                                nc.vector.tensor_copy(dst[:, m, n * 512:(n + 1) * 512],
                                                      qps[:])

            def stage4_v(c, mix8, v8):
                for t in range(NT):
                    for n in range(2):
                        vps = ps.tile([128, 512], f32, tag="mm")
                        for j in range(KH // 2):
                            nc.tensor.matmul(
                                vps[:], mix8[:, 2 * j:2 * j + 2, t * 128:(t + 1) * 128],
                                wv_sb[:, 2 * j:2 * j + 2, n * 512:(n + 1) * 512],
                                start=(j == 0), stop=(j == KH // 2 - 1),
                                perf_mode=DR)
                        nc.vector.tensor_scalar(v8[:, t, n * 512:(n + 1) * 512], vps[:],
                                                0.25, None, op0=Alu.mult)

            def scores_head(c, h, q8, k8):
                """scores + exp for one head -> et8 fp8 [128, NT, CHUNK]"""
                et8 = ws.tile([128, NT, CHUNK], f8, tag="et8", bufs=2)
                for kt in range(NT):
                    stp = ps2.tile([128, CHUNK], f32, tag="st")
                    for qn in range(2):
                        nc.tensor.matmul(
                            stp[:, qn * 512:(qn + 1) * 512],
                            k8[:, 2 * h:2 * h + 2, kt * 128:(kt + 1) * 128],
                            q8[:, 2 * h:2 * h + 2, qn * 512:(qn + 1) * 512],
                            start=True, stop=True, perf_mode=DR)
                    nc.scalar.activation(et8[:, kt, :], stp[:], Act.Exp,
                                         scale=float(HD ** -0.5 / 256.0))
                return et8

            def pv_head(c, h, et8, v8, ocat, norm_eng="dve"):
                """PV + denominator + normalize -> ocat[:, qt, h*HD:(h+1)*HD]"""
                for qt in range(NT):
                    ovp = ps.tile([128, 512], f32, tag="mm")
                    for j in range(NT // 2):
                        nc.tensor.matmul(ovp[:, :HD],
                                         et8[:, 2 * j:2 * j + 2, qt * 128:(qt + 1) * 128],
                                         v8[:, 2 * j:2 * j + 2, h * HD:(h + 1) * HD],
                                         start=(j == 0), stop=(j == NT // 2 - 1),
                                         perf_mode=DR)
                        # denominator column in the same PSUM bank; start=False
                        # always (j==0 PV's bank-clear leaves has_written=0 so
                        # the first write overwrites, later ones accumulate).
                        nc.tensor.matmul(ovp[:, HD:HD + 1],
                                         et8[:, 2 * j:2 * j + 2, qt * 128:(qt + 1) * 128],
                                         ones8_sb[:],
                                         start=False, stop=(j == NT // 2 - 1),
                                         perf_mode=DR, skip_group_check=True)
                    rq = sm.tile([128, 1], f32, tag="rq")
                    nc.vector.reciprocal(rq[:], ovp[:, HD:HD + 1])
                    if norm_eng == "dve":
                        nc.vector.tensor_scalar(ocat[:, qt, h * HD:(h + 1) * HD],
                                                ovp[:, :HD], rq[:], None, op0=Alu.mult)
                    else:
                        nc.scalar.activation(ocat[:, qt, h * HD:(h + 1) * HD],
                                             ovp[:, :HD], Act.Identity, scale=rq[:])

            def tail_p1(c, qt, ocat, mixN, res, mvA, cast_eng="pool",
                        pair_ps=False):
                """transpose o, wo matmul, residual into res, LN stats."""
                otcb = ws.tile([128, KH, 128], bf, tag="sm2b", bufs=4)
                nc.sync.dma_start_transpose(otcb[:], ocat[:, qt, :])
                otc8 = ws.tile([128, KH, 128], f8, tag="otc8", bufs=4)
                if cast_eng == "pool":
                    nc.gpsimd.tensor_copy(otc8[:], otcb[:])
                else:
                    nc.scalar.copy(otc8[:], otcb[:])
                if pair_ps:
                    # both wo halves into one 2-bank ps2 tile -> single wide stt
                    wps = ps2.tile([128, CHUNK], f32, tag="st")
                    for n in range(2):
                        for j in range(KH // 2):
                            nc.tensor.matmul(
                                wps[:, n * 512:(n + 1) * 512],
                                otc8[:, 2 * j:2 * j + 2, :],
                                wo_sb[:, 2 * j:2 * j + 2, n * 512:(n + 1) * 512],
                                start=(j == 0), stop=(j == KH // 2 - 1),
                                perf_mode=DR)
                    nc.vector.scalar_tensor_tensor(
                        res[:, qt, :], wps[:], 1.0 / 128.0,
                        mixN[:, qt, :], op0=Alu.mult, op1=Alu.add)
                else:
                    for n in range(2):
                        ops_ = ps.tile([128, 512], f32, tag="mm")
                        for j in range(KH // 2):
                            nc.tensor.matmul(ops_[:],
                                             otc8[:, 2 * j:2 * j + 2, :],
                                             wo_sb[:, 2 * j:2 * j + 2, n * 512:(n + 1) * 512],
                                             start=(j == 0), stop=(j == KH // 2 - 1),
                                             perf_mode=DR)
                        nc.vector.scalar_tensor_tensor(
                            res[:, qt, n * 512:(n + 1) * 512], ops_[:], 1.0 / 128.0,
                            mixN[:, qt, n * 512:(n + 1) * 512],
                            op0=Alu.mult, op1=Alu.add)
                st6 = sm.tile([128, 2, 6], f32, tag="st6b")
                for half in range(2):
                    nc.vector.bn_stats(st6[:, half, :],
                                       res[:, qt, half * 512:(half + 1) * 512])
                nc.vector.bn_aggr(mvA[:, qt, :], st6[:])

            def tail_iv(c, mvA, ivA, negA):
                """batched rsqrt(var+eps) for the whole chunk (one Sqrt op)."""
                sqA = sm.tile([128, NT], f32, tag="sqA")
                nc.scalar.activation(sqA[:], mvA[:, :, 1], Act.Sqrt, bias=eps_sb[:])
                nc.vector.reciprocal(ivA[:], sqA[:])
                nc.vector.tensor_scalar(negA[:], ivA[:], -1.0, None, op0=Alu.mult)

            def tail_p2(c, qt, res, mvA, ivA, negA, ych, z_eng="pool"):
                """z = res - mean; y = (z @ gw) * iv + bw."""
                z = ws.tile([128, CHUNK], bf, tag="sm2b", bufs=4, name="z")
                if z_eng == "pool":
                    nc.gpsimd.tensor_scalar(z[:], res[:, qt, :], mvA[:, qt, 0:1], None,
                                            op0=Alu.subtract)
                else:
                    # z~ = mean - res; compensated by negated iv in the stt
                    nc.scalar.activation(z[:], res[:, qt, :], Act.Identity,
                                         bias=mvA[:, qt, 0:1], scale=-1.0)
                zT = ws.tile([128, KH, 128], bf, tag="sm2b", bufs=4, name="zT")
                nc.sync.dma_start_transpose(zT[:], z[:])
                yps = ps.tile([128, 512], f32, tag="mm")
                for fi in range(KH):
                    nc.tensor.matmul(yps[:, :G], zT[:, fi, :], gw_sb[:, fi, :],
                                     start=(fi == 0), stop=(fi == KH - 1))
                dv = negA if z_eng != "pool" else ivA
                nc.vector.scalar_tensor_tensor(ych[:, qt, :], yps[:, :G],
                                               dv[:, qt:qt + 1], bw_sb[:],
                                               op0=Alu.mult, op1=Alu.add)

            def store(c, ych, t0, t1):
                nc.sync.dma_start(
                    y.ap()[c, t0 * 128:t1 * 128, :].rearrange(
                        "(t p) g -> p t g", p=128),
                    ych[:, t0:t1, :])

            # =================== emission schedule ===================
            coef0 = stage1(0)
            nc.sync.dma_start(w1a_sb[:], w1a.ap())
            nc.sync.dma_start(gln_sb[:], gln.ap())
            nc.sync.dma_start(bln_sb[:], bln.ap())
            coef1 = stage1(1)
            nc.sync.dma_start(b2_sb[:], b2c.ap())
            nc.sync.dma_start(w2_sb[:], w2.ap().rearrange("(i p) m -> p i m", p=128))

            hidT0 = ws.tile([128, KM, CHUNK], bf, tag="hidT", bufs=1)
            stage2(0, coef0, hidT0)
            load_w8(wq_sb, wq8)
            load_w8(wk_sb, wk8)
            mix8_0 = ws.tile([128, KH, CHUNK], f8, tag="mix8", bufs=1)
            mixN0 = ws.tile([128, NT, H], bf, tag="mixN", bufs=2)
            stage3(0, hidT0, mix8_0, mixN0)
            load_w8(wv_sb, wv8)

            q8_0 = ws.tile([128, KH, CHUNK], f8, tag="q8", bufs=1)
            k8_0 = ws.tile([128, KH, CHUNK], f8, tag="k8", bufs=1)
            v8_0 = ws.tile([128, NT, H], f8, tag="v8y", bufs=2)
            stage4_qk(0, mix8_0, q8_0, k8_0, range(KH))
            stage4_v(0, mix8_0, v8_0)
            if debug:
                nc.sync.dma_start(dbg["dcoef"].ap(), coef0[:])
                nc.sync.dma_start(dbg["dhidT"].ap(), hidT0[:])
                nc.sync.dma_start(dbg["dmix8"].ap(), mix8_0[:])
                nc.sync.dma_start(dbg["dmixN"].ap(), mixN0[:])
                nc.sync.dma_start(dbg["dq8"].ap(), q8_0[:])
                nc.sync.dma_start(dbg["dk8"].ap(), k8_0[:])
                nc.sync.dma_start(dbg["dv8"].ap(), v8_0[:])

            # --- c0 attention interleaved with c1 front stages ---
            ocat0 = ws.tile([128, NT, H], bf, tag="ocat", bufs=1)
            et_a = scores_head(0, 0, q8_0, k8_0)
            load_w8(wo_sb, wo8)
            nc.sync.dma_start(gw_sb[:], gw.ap().rearrange("(i p) g -> p i g", p=128))
            nc.sync.dma_start(bw_sb[:], bw.ap())
            hidT1 = ws.tile([128, KM, CHUNK], bf, tag="hidT", bufs=1)
            stage2(1, coef1, hidT1)
            et_b = scores_head(0, 1, q8_0, k8_0)
            pv_head(0, 0, et_a, v8_0, ocat0)
            if debug:
                nc.sync.dma_start(dbg["det8"].ap(), et_b[:])
            mix8_1 = ws.tile([128, KH, CHUNK], f8, tag="mix8", bufs=1)
            mixN1 = ws.tile([128, NT, H], bf, tag="mixN", bufs=2)
            stage3(1, hidT1, mix8_1, mixN1)
            et_c = scores_head(0, 2, q8_0, k8_0)
            pv_head(0, 1, et_b, v8_0, ocat0)
            et_d = scores_head(0, 3, q8_0, k8_0)
            q8_1 = ws.tile([128, KH, CHUNK], f8, tag="q8", bufs=1)
            k8_1 = ws.tile([128, KH, CHUNK], f8, tag="k8", bufs=1)
            v8_1 = ws.tile([128, NT, H], f8, tag="v8y", bufs=2)
            stage4_qk(1, mix8_1, q8_1, k8_1, range(0, 2))
            pv_head(0, 2, et_c, v8_0, ocat0)

            ych0 = ws.tile([128, NT, G], f32, tag="v8y", bufs=2, name="ych")
            et_e = scores_head(1, 0, q8_1, k8_1)
            stage4_qk(1, mix8_1, q8_1, k8_1, range(2, KH))
            stage4_v(1, mix8_1, v8_1)
            pv_head(0, 3, et_d, v8_0, ocat0)
            if debug:
                nc.sync.dma_start(dbg["docat"].ap(), ocat0[:])
            et_f = scores_head(1, 1, q8_1, k8_1)

            # --- c0 tails (two passes) with c1 PV interleaved ---
            res0 = ws.tile([128, NT, H], bf, tag="res", bufs=1)
            mvA0 = sm.tile([128, NT, 2], f32, tag="mvA", bufs=2)
            ivA0 = sm.tile([128, NT], f32, tag="ivA", bufs=2)
            negA0 = sm.tile([128, NT], f32, tag="negA", bufs=2)
            ocat1 = ws.tile([128, NT, H], bf, tag="ocat", bufs=1, name="ocat")
            for qt in range(4):
                tail_p1(0, qt, ocat0, mixN0, res0, mvA0, cast_eng="pool")
            pv_head(1, 0, et_e, v8_1, ocat1)
            for qt in range(4, NT):
                tail_p1(0, qt, ocat0, mixN0, res0, mvA0, cast_eng="pool")
            if debug:
                nc.sync.dma_start(dbg["dres"].ap(), res0[:])
            tail_iv(0, mvA0, ivA0, negA0)
            et_g = scores_head(1, 2, q8_1, k8_1)
            for qt in range(4):
                tail_p2(0, qt, res0, mvA0, ivA0, negA0, ych0, z_eng="pool")
            store(0, ych0, 0, 4)
            pv_head(1, 1, et_f, v8_1, ocat1)
            for qt in range(4, NT):
                tail_p2(0, qt, res0, mvA0, ivA0, negA0, ych0, z_eng="pool")
            store(0, ych0, 4, NT)
            et_h = scores_head(1, 3, q8_1, k8_1)
            pv_head(1, 2, et_g, v8_1, ocat1, norm_eng="act")

            # --- c1 tails (Act engine is idle here; use it) ---
            res1 = ws.tile([128, NT, H], bf, tag="res", bufs=1)
            mvA1 = sm.tile([128, NT, 2], f32, tag="mvA", bufs=2)
            ivA1 = sm.tile([128, NT], f32, tag="ivA", bufs=2)
            negA1 = sm.tile([128, NT], f32, tag="negA", bufs=2)
            ych1 = ws.tile([128, NT, G], f32, tag="v8y", bufs=2, name="ych")
            pv_head(1, 3, et_h, v8_1, ocat1, norm_eng="act")
            for qt in range(NT):
                tail_p1(1, qt, ocat1, mixN1, res1, mvA1, cast_eng="pool",
                        pair_ps=True)
            tail_iv(1, mvA1, ivA1, negA1)
            for qt in range(NT):
                tail_p2(1, qt, res1, mvA1, ivA1, negA1, ych1, z_eng="act")
                if qt == 3:
                    store(1, ych1, 0, 4)
            store(1, ych1, 4, NT)

    nc.compile()
    return nc


def _get_compiled(debug=False):
    global _COMPILED
    if _COMPILED is None:
        _COMPILED = _build(debug=debug)
    return _COMPILED


def _f8c(x):
    return np.clip(x, -240.0, 240.0).astype(F8)


def _prep_inputs(inputs):
    f32 = np.float32

    def a(name):
        return np.asarray(inputs[name], dtype=f32)

    x = a("x")
    mw = a("mother_wavelets")
    scales = a("scales")
    norm = np.sqrt(np.sum(mw ** 2, axis=2, keepdims=True))
    kern = (mw / np.maximum(norm, 1e-12)) * (1.0 / (1.0 + np.exp(-scales)))
    kern = kern[0, :, :, 0]                      # (W, H)
    kernT = np.ascontiguousarray(kern.T).astype(BF16)

    w1a = np.concatenate([a("mix_w1"), a("mix_b1")[None, :]], axis=0).astype(BF16)
    gln = np.ascontiguousarray(a("mix_ln_g").reshape(KM, 128).T).astype(f32)
    bln = np.ascontiguousarray(a("mix_ln_b").reshape(KM, 128).T).astype(f32)
    w2 = a("mix_w2").astype(BF16)
    b2c = np.ascontiguousarray(a("mix_b2").reshape(KH, 128).T).astype(f32)
    gw = (a("out_ln_g")[:, None] * a("out_w")).astype(BF16)
    bw_vec = a("out_ln_b") @ a("out_w") + a("out_b")
    bw = np.tile(bw_vec[None, :], (128, 1)).astype(f32)

    shared = {
        "kernt": kernT, "w1a": w1a, "gln": gln, "bln": bln, "w2": w2,
        "b2c": b2c,
        "wq8": _f8c(16.0 * a("wq")), "wk8": _f8c(16.0 * a("wk")),
        "wv8": _f8c(16.0 * a("wv")), "wo8": _f8c(16.0 * a("wo")),
        "gw": gw, "bw": bw,
    }

    xc = x.reshape(N_CHUNKS, CHUNK, H)
    # xt[c, ki, p, t] = xc[c, t, ki*128+p]  (feature-major per 128-slice)
    xt_all = np.ascontiguousarray(
        xc.transpose(0, 2, 1).reshape(N_CHUNKS, KH, 128, CHUNK)).astype(BF16)
    in_maps = []
    for core in range(N_CORES):
        m = dict(shared)
        m["xt"] = np.ascontiguousarray(xt_all[core * CPC:(core + 1) * CPC])
        in_maps.append(m)
    return in_maps


def kernel(**inputs) -> np.ndarray:
    from concourse.bass_utils import run_bass_kernel_spmd

    nc = _get_compiled()
    in_maps = _prep_inputs(inputs)
    res = run_bass_kernel_spmd(nc, in_maps, core_ids=list(range(N_CORES)))
    out = np.concatenate([r["y"] for r in res.results], axis=0)  # (16, CHUNK, G)
    return out.reshape(B, S, G).astype(np.float32)
